# revision 3
# baseline (speedup 1.0000x reference)
"""Trainium2 Bass kernel for nn_Projection: out = [(1-s)*x, s],
s = -(1-||x||^2)/(1+||x||^2) per row.

Identity used: with sq = sum(x^2), s = (sq-1)/(sq+1) = 1 - 2/(1+sq).
Let t = 2/(1+sq). Then out = [t*x, 1-t].

HBM-bandwidth bound (elementwise over 512MB in / 516MB out). Gate is
rel_err < 2e-2, so all HBM traffic is bf16 (measured max rel err
~1.2e-2): the host rounds x to bf16, the device computes and stores
bf16, the host upcasts to f32. Halves HBM traffic vs f32.

Engine budget per 524288-elem tile (blk=32, ~5.9us DMA):
  ACT   : square (bf16->fp16, ~3.7us) + s-column (skewed 1 iter)
  DVE   : fold-tree row-sum in fp16 2x perf mode (3 folds + short
          reduce, ~2.4us) + recip chain + 44% of the t*x multiply
  GpSimd: 56% of the t*x multiply (~2 cyc/elem two-input floor)
The s column accumulates in SBUF ([128, K] bf16) and is stored once
at the end, so hot-loop stores are the pure [*, 128] bf16 tensor.

Sharding: pure data parallel over rows across 8 NeuronCores.
Per-core layout: partition p owns rows [p*K, (p+1)*K), K = R/128.
"""

import sys

for _p in ("/opt/trn_rl_repo", "/opt/trn_rl_repo/concourse"):
    if _p not in sys.path:
        sys.path.insert(0, _p)

import ml_dtypes
import numpy as np

import concourse.bacc as bacc
import concourse.tile as tile
from concourse import mybir
from concourse.bass_utils import run_bass_kernel_spmd

N, D = 1048576, 128
N_CORES = 8
R = N // N_CORES   # 131072 rows per core
P = 128            # SBUF partitions
K = R // P         # 1024 rows per partition
BF16 = mybir.dt.bfloat16
FP16 = mybir.dt.float16
F32 = mybir.dt.float32
NP_BF16 = np.dtype(ml_dtypes.bfloat16)


def build_nc(rows: int = R, blk: int = 32, split: int = 72,
             io_bufs: int = 6, tmp_bufs: int = 2):
    """Per-core Bass program: x[rows, D] bf16 -> tx[rows, D] bf16,
    s[P, rows//P] bf16 (s for row p*K+k lives at s[p, k]).
    split: d-range [0:split) of the multiply runs on GpSimd, rest on DVE."""
    k_rows = rows // P
    assert k_rows % blk == 0
    niter = k_rows // blk

    nc = bacc.Bacc(trn_type="TRN2")
    x = nc.dram_tensor("x", [rows, D], BF16, kind="ExternalInput")
    tx = nc.dram_tensor("tx", [rows, D], BF16, kind="ExternalOutput")
    s = nc.dram_tensor("s", [P, k_rows], BF16, kind="ExternalOutput")

    # row = p*k_rows + i*blk + j: each partition's chunk per iteration is
    # blk consecutive rows = one contiguous blk*256B DMA span.
    xv = x.ap().rearrange("(p c j) d -> c p j d", p=P, j=blk)
    tv = tx.ap().rearrange("(p c j) d -> c p j d", p=P, j=blk)

    PRE = min(4, niter)  # load prefetch distance

    with tile.TileContext(nc) as tc:
        with (
            tc.tile_pool(name="io", bufs=io_bufs) as io_pool,
            tc.tile_pool(name="tmp", bufs=tmp_bufs) as tmp_pool,
            tc.tile_pool(name="small", bufs=8) as small_pool,
            tc.tile_pool(name="singles", bufs=1) as singles,
        ):
            half = singles.tile([P, 1], F32)
            nc.vector.memset(half, 0.5)
            half_b = half[:, 0:1].broadcast_to([P, blk])

            s_all = singles.tile([P, k_rows], BF16)

            pending = []

            def issue_load(i):
                # Loads on the ACT HWDGE ring; stores on SP's ring, so the
                # two drain concurrently (one ring serializes its DMAs).
                x_t = io_pool.tile([P, blk, D], BF16, tag="x")
                nc.scalar.dma_start(out=x_t, in_=xv[i])
                pending.append(x_t)

            for i in range(PRE):
                issue_load(i)

            t_prev = None  # t32 of iteration i-1, for the skewed s-column

            for i in range(niter):
                if i + PRE < niter:
                    issue_load(i + PRE)
                x_t = pending.pop(0)

                # xsq = (x/sqrt(2))^2 = x^2/2 on ACT, fp16 out. The 1/2
                # folds the final *2: t = 2/(1+sum x^2) = 1/(0.5+sum x^2/2).
                xsq = tmp_pool.tile([P, blk, D], FP16, tag="xsq")
                nc.scalar.activation(
                    out=xsq, in_=x_t,
                    func=mybir.ActivationFunctionType.Square,
                    scale=0.7071067811865476,
                )

                # s column of iteration i-1 (skewed so ACT never stalls
                # waiting on this iteration's DVE chain): 1 - t.
                if t_prev is not None:
                    nc.scalar.activation(
                        out=s_all[:, (i - 1) * blk:i * blk], in_=t_prev,
                        func=mybir.ActivationFunctionType.Copy,
                        bias=1.0, scale=-1.0,
                    )

                # Row-sum of xsq via fp16 fold tree (each fold is a 2-byte
                # contiguous tensor_tensor -> DVE 2x perf mode), then a
                # short 16-element reduce accumulating in f32.
                f1 = tmp_pool.tile([P, blk, 64], FP16, tag="f1")
                nc.vector.tensor_add(f1, xsq[:, :, 0:64], xsq[:, :, 64:128])
                f2 = tmp_pool.tile([P, blk, 32], FP16, tag="f2")
                nc.vector.tensor_add(f2, f1[:, :, 0:32], f1[:, :, 32:64])
                f3 = tmp_pool.tile([P, blk, 16], FP16, tag="f3")
                nc.vector.tensor_add(f3, f2[:, :, 0:16], f2[:, :, 16:32])
                sq = small_pool.tile([P, blk], F32, tag="sq")
                nc.vector.reduce_sum(out=sq, in_=f3, axis=mybir.AxisListType.X)

                u = small_pool.tile([P, blk], F32, tag="u")
                nc.vector.tensor_add(u, sq, half_b)
                t32 = small_pool.tile([P, blk], F32, tag="t32")
                nc.vector.reciprocal(out=t32, in_=u)
                t16 = small_pool.tile([P, blk], BF16, tag="t16")
                nc.vector.tensor_copy(t16, t32)
                t_prev = t32

                out_t = io_pool.tile([P, blk, D], BF16, tag="out")
                t_b = t16[:, :].unsqueeze(2)
                if split > 0:
                    nc.gpsimd.tensor_mul(
                        out_t[:, :, 0:split], x_t[:, :, 0:split],
                        t_b.broadcast_to([P, blk, split]))
                if split < D:
                    nc.vector.tensor_mul(
                        out_t[:, :, split:D], x_t[:, :, split:D],
                        t_b.broadcast_to([P, blk, D - split]))

                nc.sync.dma_start(out=tv[i], in_=out_t)

            # last iteration's s column + one contiguous 2KB/partition store
            nc.scalar.activation(
                out=s_all[:, (niter - 1) * blk:niter * blk], in_=t_prev,
                func=mybir.ActivationFunctionType.Copy,
                bias=1.0, scale=-1.0,
            )
            nc.scalar.dma_start(out=s.ap(), in_=s_all)

    nc.compile()
    return nc


_nc_cache: dict = {}


def _get_nc(rows: int = R, blk: int = 32, split: int = 72):
    key = (rows, blk, split)
    if key not in _nc_cache:
        _nc_cache[key] = build_nc(rows, blk, split)
    return _nc_cache[key]


def kernel(x) -> np.ndarray:
    x = np.asarray(x)
    assert x.shape == (N, D), x.shape
    x16 = np.ascontiguousarray(x.astype(NP_BF16))
    nc = _get_nc()
    shards = x16.reshape(N_CORES, R, D)
    in_maps = [{"x": shards[c]} for c in range(N_CORES)]
    res = run_bass_kernel_spmd(nc, in_maps, core_ids=list(range(N_CORES)))
    out = np.empty((N, D + 1), dtype=np.float32)
    for c, r in enumerate(res.results):
        out[c * R:(c + 1) * R, :D] = r["tx"].astype(np.float32)
        out[c * R:(c + 1) * R, D] = r["s"].reshape(R).astype(np.float32)
    return out


# revision 5
# speedup vs baseline: 1.5999x; 1.5999x over previous
"""Trainium2 Bass kernel for nn_Projection: out = [(1-s)*x, s],
s = -(1-||x||^2)/(1+||x||^2) per row.

Identity used: with sq = sum(x^2), s = (sq-1)/(sq+1) = 1 - 2/(1+sq).
Let t = 2/(1+sq). Then out = [t*x, 1-t].

HBM-bandwidth bound (elementwise over 512MB in / 516MB out). Gate is
rel_err < 2e-2, so all HBM traffic is bf16 (measured max rel err
~1.2e-2): the host rounds x to bf16, the device computes and stores
bf16, the host upcasts to f32. Halves HBM traffic vs f32.

Layout trick: tiles are d-major in SBUF ([P, D, blk], host pre/post
transposes the per-tile element order). This makes every hot DVE op
eligible for the 2-byte packed 2x perf mode (innermost step 1 on all
operands, including the per-row t broadcast which is stride-0 only
in the middle dim):
  - row-sum of x^2: 7-level fp16 fold tree over d, each fold a 2x TT
  - t*x multiply: one 2x TT with t16 broadcast over d
GpSimd stays idle: any GpSimd op would serialize with DVE 2x ops on
the exclusive shared SBUF port pair.

Software-pipelined with a 2-iteration skew so the in-order ACT/DVE
queues never stall on each other:
  ACT iter k: u_{k-2}=sq+0.5, s_col_{k-3}, square_k   (+ load k+PRE)
  DVE iter k: folds_{k-1}, recip_{k-2}, cast_{k-2}, mul_{k-2}
  SP  iter k: store_{k-2}

Sharding: pure data parallel over rows across 8 NeuronCores.
Per-core row map: row = p*K + i*blk + j  (p partition, i iteration).
"""

import sys

for _p in ("/opt/trn_rl_repo", "/opt/trn_rl_repo/concourse"):
    if _p not in sys.path:
        sys.path.insert(0, _p)

import ml_dtypes
import numpy as np

import concourse.bacc as bacc
import concourse.tile as tile
from concourse import mybir
from concourse.bass_utils import run_bass_kernel_spmd

N, D = 1048576, 128
N_CORES = 8
R = N // N_CORES   # 131072 rows per core
P = 128            # SBUF partitions
K = R // P         # 1024 rows per partition
BLK = 32
NITER = K // BLK
BF16 = mybir.dt.bfloat16
FP16 = mybir.dt.float16
F32 = mybir.dt.float32
NP_BF16 = np.dtype(ml_dtypes.bfloat16)


def build_nc(blk: int = BLK, pre: int = 3):
    k_rows = K
    niter = k_rows // blk
    fd = blk * D

    nc = bacc.Bacc(trn_type="TRN2")
    x = nc.dram_tensor("x", [niter, P, fd], BF16, kind="ExternalInput")
    tx = nc.dram_tensor("tx", [niter, P, fd], BF16, kind="ExternalOutput")
    s = nc.dram_tensor("s", [P, k_rows], BF16, kind="ExternalOutput")
    xv = x.ap()
    tv = tx.ap()

    with tile.TileContext(nc) as tc:
        with (
            tc.tile_pool(name="xin", bufs=pre + 4) as x_pool,
            tc.tile_pool(name="out", bufs=3) as o_pool,
            tc.tile_pool(name="sqp", bufs=3) as sq_pool,
            tc.tile_pool(name="fld", bufs=2) as f_pool,
            tc.tile_pool(name="small", bufs=4) as small_pool,
            tc.tile_pool(name="singles", bufs=1) as singles,
        ):
            s_all = singles.tile([P, k_rows], BF16)

            x_t = {}     # j -> x tile
            xsq = {}     # j -> squared tile
            sqs = {}     # j -> row-sum (fp16 [P, blk])
            us = {}      # j -> u = sq + 0.5 (f32)
            t32s = {}    # j -> 1/u (f32)

            def load(j):
                x_t[j] = x_pool.tile([P, D, blk], BF16, tag="x", name="x_t")
                nc.scalar.dma_start(out=x_t[j], in_=xv[j])

            def square(j):
                xsq[j] = sq_pool.tile([P, D, blk], FP16, tag="xsq", name="xsq")
                nc.scalar.activation(
                    out=xsq[j], in_=x_t[j],
                    func=mybir.ActivationFunctionType.Square,
                    scale=0.7071067811865476,
                )

            def folds(j):
                a = xsq[j]
                d = D
                lvl = 0
                while d > 2:
                    d //= 2
                    lvl += 1
                    b = f_pool.tile([P, d, blk], FP16, tag=f"f{lvl}", name=f"f{lvl}")
                    nc.vector.tensor_add(b, a[:, 0:d, :], a[:, d:2 * d, :])
                    a = b
                sqs[j] = small_pool.tile([P, blk], FP16, tag="sq", name="sq")
                nc.vector.tensor_add(
                    sqs[j][:, :].unsqueeze(1), a[:, 0:1, :], a[:, 1:2, :])
                del xsq[j]

            def u_of(j):
                us[j] = small_pool.tile([P, blk], F32, tag="u", name="u")
                nc.scalar.activation(
                    out=us[j], in_=sqs[j],
                    func=mybir.ActivationFunctionType.Copy, bias=0.5,
                )
                del sqs[j]

            def tail(j):
                t32s[j] = small_pool.tile([P, blk], F32, tag="t32", name="t32")
                nc.vector.reciprocal_approx_fast(out=t32s[j], in_=us[j])
                del us[j]
                t16 = small_pool.tile([P, blk], BF16, tag="t16")
                nc.vector.tensor_copy(t16, t32s[j])
                out_t = o_pool.tile([P, D, blk], BF16, tag="out")
                nc.vector.tensor_mul(
                    out_t, x_t[j],
                    t16[:, :].unsqueeze(1).broadcast_to([P, D, blk]))
                del x_t[j]
                nc.sync.dma_start(out=tv[j], in_=out_t)

            def s_col(j):
                nc.scalar.activation(
                    out=s_all[:, j * blk:(j + 1) * blk], in_=t32s[j],
                    func=mybir.ActivationFunctionType.Copy,
                    bias=1.0, scale=-1.0,
                )
                del t32s[j]

            for j in range(pre):
                load(j)

            for k in range(niter + 3):
                # ACT: smalls first so they never queue behind the square
                if 0 <= k - 2 < niter:
                    u_of(k - 2)
                if 0 <= k - 3 < niter:
                    s_col(k - 3)
                if k < niter:
                    if k + pre < niter:
                        load(k + pre)
                    square(k)
                # DVE
                if 0 <= k - 1 < niter:
                    folds(k - 1)
                if 0 <= k - 2 < niter:
                    tail(k - 2)

            nc.scalar.dma_start(out=s.ap(), in_=s_all)

    nc.compile()
    return nc


def host_pack(x16_flat: np.ndarray) -> np.ndarray:
    """[N_CORES*R, D] bf16 row-major -> [N_CORES, NITER, P, D*BLK] d-major."""
    v = x16_flat.view(np.uint16)
    v = v.reshape(N_CORES, P, NITER, BLK, D)
    v = np.ascontiguousarray(v.transpose(0, 2, 1, 4, 3))
    return v.reshape(N_CORES, NITER, P, D * BLK).view(NP_BF16)


def host_unpack_tx(txd: np.ndarray) -> np.ndarray:
    """[NITER, P, D*BLK] d-major -> [R, D] f32."""
    v = txd.view(np.uint16).reshape(NITER, P, D, BLK)
    v = np.ascontiguousarray(v.transpose(1, 0, 3, 2))
    return v.reshape(R, D).view(NP_BF16).astype(np.float32)


_nc_cache: dict = {}


def _get_nc():
    if "nc" not in _nc_cache:
        _nc_cache["nc"] = build_nc()
    return _nc_cache["nc"]


def kernel(x) -> np.ndarray:
    x = np.asarray(x)
    assert x.shape == (N, D), x.shape
    x16 = np.ascontiguousarray(x.astype(NP_BF16))
    packed = host_pack(x16)
    nc = _get_nc()
    in_maps = [{"x": packed[c]} for c in range(N_CORES)]
    res = run_bass_kernel_spmd(nc, in_maps, core_ids=list(range(N_CORES)))
    out = np.empty((N, D + 1), dtype=np.float32)
    for c, r in enumerate(res.results):
        out[c * R:(c + 1) * R, :D] = host_unpack_tx(r["tx"])
        out[c * R:(c + 1) * R, D] = r["s"].reshape(R).astype(np.float32)
    return out


# revision 8
# speedup vs baseline: 1.6187x; 1.0117x over previous
"""Trainium2 Bass kernel for nn_Projection: out = [(1-s)*x, s],
s = -(1-||x||^2)/(1+||x||^2) per row.

Identity used: with sq = sum(x^2), s = (sq-1)/(sq+1) = 1 - 2/(1+sq).
Let t = 2/(1+sq). Then out = [t*x, 1-t].

HBM-bandwidth bound (elementwise over 512MB in / 516MB out). Gate is
rel_err < 2e-2, so all HBM traffic is bf16 (measured max rel err
~1.2e-2): the host rounds x to bf16, the device computes and stores
bf16, the host upcasts to f32. Halves HBM traffic vs f32.

Layout trick: tiles are d-major in SBUF ([P, D, blk], host pre/post
transposes the per-tile element order). This makes every hot DVE op
eligible for the 2-byte packed 2x perf mode (innermost step 1 on all
operands, including the per-row t broadcast which is stride-0 only
in the middle dim):
  - row-sum of x^2: 7-level fp16 fold tree over d, each fold a 2x TT
  - t*x multiply: one 2x TT with t16 broadcast over d
GpSimd stays idle: any GpSimd op would serialize with DVE 2x ops on
the exclusive shared SBUF port pair.

Software-pipelined with a 2-iteration skew so the in-order ACT/DVE
queues never stall on each other:
  ACT iter k: u_{k-2}=sq+0.5, s_col_{k-3}, square_k   (+ load k+PRE)
  DVE iter k: folds_{k-1}, recip_{k-2}, cast_{k-2}, mul_{k-2}
  SP  iter k: store_{k-2}

Sharding: pure data parallel over rows across 8 NeuronCores.
Per-core row map: row = p*K + i*blk + j  (p partition, i iteration).
"""

import sys

for _p in ("/opt/trn_rl_repo", "/opt/trn_rl_repo/concourse"):
    if _p not in sys.path:
        sys.path.insert(0, _p)

import ml_dtypes
import numpy as np

import concourse.bacc as bacc
import concourse.tile as tile
from concourse import mybir
from concourse.bass_utils import run_bass_kernel_spmd

N, D = 1048576, 128
N_CORES = 8
R = N // N_CORES   # 131072 rows per core
P = 128            # SBUF partitions
K = R // P         # 1024 rows per partition
BLK = 32
NITER = K // BLK
BF16 = mybir.dt.bfloat16
FP16 = mybir.dt.float16
F32 = mybir.dt.float32
NP_BF16 = np.dtype(ml_dtypes.bfloat16)


def build_nc(blk: int = BLK, pre: int = 6):
    k_rows = K
    niter = k_rows // blk
    fd = blk * D

    nc = bacc.Bacc(trn_type="TRN2")
    x = nc.dram_tensor("x", [niter, P, fd], BF16, kind="ExternalInput")
    tx = nc.dram_tensor("tx", [niter, P, fd], BF16, kind="ExternalOutput")
    s = nc.dram_tensor("s", [P, k_rows], BF16, kind="ExternalOutput")
    xv = x.ap()
    tv = tx.ap()

    with tile.TileContext(nc) as tc:
        with (
            tc.tile_pool(name="xin", bufs=pre + 4) as x_pool,
            tc.tile_pool(name="out", bufs=4) as o_pool,
            tc.tile_pool(name="sqp", bufs=3) as sq_pool,
            tc.tile_pool(name="fld", bufs=2) as f_pool,
            tc.tile_pool(name="small", bufs=4) as small_pool,
            tc.tile_pool(name="singles", bufs=1) as singles,
        ):
            s_all = singles.tile([P, k_rows], BF16)

            x_t = {}     # j -> x tile
            xsq = {}     # j -> squared tile
            sqs = {}     # j -> row-sum (fp16 [P, blk])
            us = {}      # j -> u = sq + 0.5 (f32)
            t32s = {}    # j -> 1/u (f32)

            def load(j):
                x_t[j] = x_pool.tile([P, D, blk], BF16, tag="x", name="x_t")
                nc.scalar.dma_start(out=x_t[j], in_=xv[j])

            def square(j):
                xsq[j] = sq_pool.tile([P, D, blk], FP16, tag="xsq", name="xsq")
                nc.scalar.activation(
                    out=xsq[j], in_=x_t[j],
                    func=mybir.ActivationFunctionType.Square,
                    scale=0.7071067811865476,
                )

            def folds(j):
                a = xsq[j]
                d = D
                lvl = 0
                while d > 2:
                    d //= 2
                    lvl += 1
                    b = f_pool.tile([P, d, blk], FP16, tag=f"f{lvl}", name=f"f{lvl}")
                    nc.vector.tensor_add(b, a[:, 0:d, :], a[:, d:2 * d, :])
                    a = b
                sqs[j] = small_pool.tile([P, blk], FP16, tag="sq", name="sq")
                nc.vector.tensor_add(
                    sqs[j][:, :].unsqueeze(1), a[:, 0:1, :], a[:, 1:2, :])
                del xsq[j]

            def u_of(j):
                us[j] = small_pool.tile([P, blk], F32, tag="u", name="u")
                nc.scalar.activation(
                    out=us[j], in_=sqs[j],
                    func=mybir.ActivationFunctionType.Copy, bias=0.5,
                )
                del sqs[j]

            def tail(j):
                t32s[j] = small_pool.tile([P, blk], F32, tag="t32", name="t32")
                nc.vector.reciprocal_approx_fast(out=t32s[j], in_=us[j])
                del us[j]
                t16 = small_pool.tile([P, blk], BF16, tag="t16")
                nc.vector.tensor_copy(t16, t32s[j])
                out_t = o_pool.tile([P, D, blk], BF16, tag="out")
                nc.vector.tensor_mul(
                    out_t, x_t[j],
                    t16[:, :].unsqueeze(1).broadcast_to([P, D, blk]))
                del x_t[j]
                nc.sync.dma_start(out=tv[j], in_=out_t)

            def s_col(j):
                nc.scalar.activation(
                    out=s_all[:, j * blk:(j + 1) * blk], in_=t32s[j],
                    func=mybir.ActivationFunctionType.Copy,
                    bias=1.0, scale=-1.0,
                )
                del t32s[j]

            for j in range(pre):
                load(j)

            for k in range(niter + 3):
                # ACT: smalls first so they never queue behind the square
                if 0 <= k - 2 < niter:
                    u_of(k - 2)
                if 0 <= k - 3 < niter:
                    s_col(k - 3)
                if k < niter:
                    if k + pre < niter:
                        load(k + pre)
                    square(k)
                # DVE: tail (mul+store) first so the store DMA issues early
                # in the iteration; folds of the next tile follow.
                if 0 <= k - 2 < niter:
                    tail(k - 2)
                if 0 <= k - 1 < niter:
                    folds(k - 1)

            nc.scalar.dma_start(out=s.ap(), in_=s_all)

    nc.compile()
    return nc


def host_pack(x16_flat: np.ndarray) -> np.ndarray:
    """[N_CORES*R, D] bf16 row-major -> [N_CORES, NITER, P, D*BLK] d-major."""
    v = x16_flat.view(np.uint16)
    v = v.reshape(N_CORES, P, NITER, BLK, D)
    v = np.ascontiguousarray(v.transpose(0, 2, 1, 4, 3))
    return v.reshape(N_CORES, NITER, P, D * BLK).view(NP_BF16)


def host_unpack_tx(txd: np.ndarray) -> np.ndarray:
    """[NITER, P, D*BLK] d-major -> [R, D] f32."""
    v = txd.view(np.uint16).reshape(NITER, P, D, BLK)
    v = np.ascontiguousarray(v.transpose(1, 0, 3, 2))
    return v.reshape(R, D).view(NP_BF16).astype(np.float32)


_nc_cache: dict = {}


def _get_nc():
    if "nc" not in _nc_cache:
        _nc_cache["nc"] = build_nc()
    return _nc_cache["nc"]


def kernel(x) -> np.ndarray:
    x = np.asarray(x)
    assert x.shape == (N, D), x.shape
    x16 = np.ascontiguousarray(x.astype(NP_BF16))
    packed = host_pack(x16)
    nc = _get_nc()
    in_maps = [{"x": packed[c]} for c in range(N_CORES)]
    res = run_bass_kernel_spmd(nc, in_maps, core_ids=list(range(N_CORES)))
    out = np.empty((N, D + 1), dtype=np.float32)
    for c, r in enumerate(res.results):
        out[c * R:(c + 1) * R, :D] = host_unpack_tx(r["tx"])
        out[c * R:(c + 1) * R, D] = r["s"].reshape(R).astype(np.float32)
    return out


# revision 11
# speedup vs baseline: 1.6307x; 1.0075x over previous
"""Trainium2 Bass kernel for nn_Projection: out = [(1-s)*x, s],
s = -(1-||x||^2)/(1+||x||^2) per row.

Identity used: with sq = sum(x^2), s = (sq-1)/(sq+1) = 1 - 2/(1+sq).
Let t = 2/(1+sq). Then out = [t*x, 1-t].

HBM-bandwidth bound (elementwise over 512MB in / 516MB out). Gate is
rel_err < 2e-2, so all HBM traffic is bf16 (measured max rel err
~1.2e-2): the host rounds x to bf16, the device computes and stores
bf16, the host upcasts to f32. Halves HBM traffic vs f32.

Layout trick: tiles are d-major in SBUF ([P, D, blk], host pre/post
transposes the per-tile element order). This makes every hot DVE op
eligible for the 2-byte packed 2x perf mode (innermost step 1 on all
operands, including the per-row t broadcast which is stride-0 only
in the middle dim):
  - row-sum of x^2: 7-level fp16 fold tree over d, each fold a 2x TT
  - t*x multiply: one 2x TT with t16 broadcast over d
GpSimd stays idle: any GpSimd op would serialize with DVE 2x ops on
the exclusive shared SBUF port pair.

Software-pipelined with a 2-iteration skew so the in-order ACT/DVE
queues never stall on each other:
  ACT iter k: u_{k-2}=sq+0.5, s_col_{k-3}, square_k   (+ load k+PRE)
  DVE iter k: folds_{k-1}, recip_{k-2}, cast_{k-2}, mul_{k-2}
  SP  iter k: store_{k-2}

Sharding: pure data parallel over rows across 8 NeuronCores.
Per-core row map: row = p*K + i*blk + j  (p partition, i iteration).
"""

import sys

for _p in ("/opt/trn_rl_repo", "/opt/trn_rl_repo/concourse"):
    if _p not in sys.path:
        sys.path.insert(0, _p)

import ml_dtypes
import numpy as np

import concourse.bacc as bacc
import concourse.tile as tile
from concourse import mybir
from concourse.bass_utils import run_bass_kernel_spmd

N, D = 1048576, 128
N_CORES = 8
R = N // N_CORES   # 131072 rows per core
P = 128            # SBUF partitions
K = R // P         # 1024 rows per partition
BLK = 32
NITER = K // BLK
BF16 = mybir.dt.bfloat16
FP16 = mybir.dt.float16
F32 = mybir.dt.float32
NP_BF16 = np.dtype(ml_dtypes.bfloat16)


def build_nc(blk: int = BLK, pre: int = 6):
    k_rows = K
    niter = k_rows // blk
    fd = blk * D

    nc = bacc.Bacc(trn_type="TRN2")
    x = nc.dram_tensor("x", [niter, P, fd], BF16, kind="ExternalInput")
    tx = nc.dram_tensor("tx", [niter, P, fd], BF16, kind="ExternalOutput")
    s = nc.dram_tensor("s", [P, k_rows], BF16, kind="ExternalOutput")
    xv = x.ap()
    tv = tx.ap()

    with tile.TileContext(nc) as tc:
        with (
            tc.tile_pool(name="xin", bufs=pre + 4) as x_pool,
            tc.tile_pool(name="out", bufs=4) as o_pool,
            tc.tile_pool(name="sqp", bufs=3) as sq_pool,
            tc.tile_pool(name="fld", bufs=2) as f_pool,
            tc.tile_pool(name="small", bufs=4) as small_pool,
            tc.tile_pool(name="singles", bufs=1) as singles,
        ):
            s_all = singles.tile([P, k_rows], BF16)

            x_t = {}     # j -> x tile
            xsq = {}     # j -> squared tile
            sqs = {}     # j -> row-sum (fp16 [P, blk])
            us = {}      # j -> u = sq + 0.5 (f32)
            t32s = {}    # j -> 1/u (f32)

            def load(j):
                # Alternate rings by parity: during ramp (loads only) and
                # drain (stores only) the single active direction can then
                # use BOTH HWDGE rings (~one ring alone caps at ~240GB/s).
                x_t[j] = x_pool.tile([P, D, blk], BF16, tag="x", name="x_t")
                eng = nc.scalar if j % 2 == 0 else nc.sync
                eng.dma_start(out=x_t[j], in_=xv[j])

            def square(j):
                xsq[j] = sq_pool.tile([P, D, blk], FP16, tag="xsq", name="xsq")
                nc.scalar.activation(
                    out=xsq[j], in_=x_t[j],
                    func=mybir.ActivationFunctionType.Square,
                    scale=0.7071067811865476,
                )

            def folds(j):
                a = xsq[j]
                d = D
                lvl = 0
                while d > 2:
                    d //= 2
                    lvl += 1
                    b = f_pool.tile([P, d, blk], FP16, tag=f"f{lvl}", name=f"f{lvl}")
                    nc.vector.tensor_add(b, a[:, 0:d, :], a[:, d:2 * d, :])
                    a = b
                sqs[j] = small_pool.tile([P, blk], FP16, tag="sq", name="sq")
                nc.vector.tensor_add(
                    sqs[j][:, :].unsqueeze(1), a[:, 0:1, :], a[:, 1:2, :])
                del xsq[j]

            def u_of(j):
                us[j] = small_pool.tile([P, blk], F32, tag="u", name="u")
                nc.scalar.activation(
                    out=us[j], in_=sqs[j],
                    func=mybir.ActivationFunctionType.Copy, bias=0.5,
                )
                del sqs[j]

            def tail(j):
                t32s[j] = small_pool.tile([P, blk], F32, tag="t32", name="t32")
                nc.vector.reciprocal_approx_fast(out=t32s[j], in_=us[j])
                del us[j]
                t16 = small_pool.tile([P, blk], BF16, tag="t16")
                nc.vector.tensor_copy(t16, t32s[j])
                out_t = o_pool.tile([P, D, blk], BF16, tag="out")
                nc.vector.tensor_mul(
                    out_t, x_t[j],
                    t16[:, :].unsqueeze(1).broadcast_to([P, D, blk]))
                del x_t[j]
                eng = nc.sync if j % 2 == 0 else nc.scalar
                eng.dma_start(out=tv[j], in_=out_t)

            def s_col(j):
                nc.scalar.activation(
                    out=s_all[:, j * blk:(j + 1) * blk], in_=t32s[j],
                    func=mybir.ActivationFunctionType.Copy,
                    bias=1.0, scale=-1.0,
                )
                del t32s[j]

            for j in range(pre):
                load(j)

            for k in range(niter + 3):
                # ACT: smalls first so they never queue behind the square
                if 0 <= k - 2 < niter:
                    u_of(k - 2)
                if 0 <= k - 3 < niter:
                    s_col(k - 3)
                if k < niter:
                    if k + pre < niter:
                        load(k + pre)
                    square(k)
                # DVE: tail (mul+store) first so the store DMA issues early
                # in the iteration; folds of the next tile follow.
                if 0 <= k - 2 < niter:
                    tail(k - 2)
                if 0 <= k - 1 < niter:
                    folds(k - 1)

            nc.sync.dma_start(out=s.ap(), in_=s_all)

    nc.compile()
    return nc


def host_pack(x16_flat: np.ndarray) -> np.ndarray:
    """[N_CORES*R, D] bf16 row-major -> [N_CORES, NITER, P, D*BLK] d-major."""
    v = x16_flat.view(np.uint16)
    v = v.reshape(N_CORES, P, NITER, BLK, D)
    v = np.ascontiguousarray(v.transpose(0, 2, 1, 4, 3))
    return v.reshape(N_CORES, NITER, P, D * BLK).view(NP_BF16)


def host_unpack_tx(txd: np.ndarray) -> np.ndarray:
    """[NITER, P, D*BLK] d-major -> [R, D] f32."""
    v = txd.view(np.uint16).reshape(NITER, P, D, BLK)
    v = np.ascontiguousarray(v.transpose(1, 0, 3, 2))
    return v.reshape(R, D).view(NP_BF16).astype(np.float32)


_nc_cache: dict = {}


def _get_nc():
    if "nc" not in _nc_cache:
        _nc_cache["nc"] = build_nc()
    return _nc_cache["nc"]


def kernel(x) -> np.ndarray:
    x = np.asarray(x)
    assert x.shape == (N, D), x.shape
    x16 = np.ascontiguousarray(x.astype(NP_BF16))
    packed = host_pack(x16)
    nc = _get_nc()
    in_maps = [{"x": packed[c]} for c in range(N_CORES)]
    res = run_bass_kernel_spmd(nc, in_maps, core_ids=list(range(N_CORES)))
    out = np.empty((N, D + 1), dtype=np.float32)
    for c, r in enumerate(res.results):
        out[c * R:(c + 1) * R, :D] = host_unpack_tx(r["tx"])
        out[c * R:(c + 1) * R, D] = r["s"].reshape(R).astype(np.float32)
    return out
